# revision 12
# baseline (speedup 1.0000x reference)
"""Trainium2 Bass kernel for a top-2 MoE layer (T=2048, H=2048, I=1408, E=8).

Strategy: expert-parallel over 8 NeuronCores. The host dispatches tokens:
for each expert e it gathers the tokens routed to e (~480 of 2048, padded
to a shared capacity C sized to the busiest expert), so each core runs a
dense [C,H] FFN for its expert. The host then combines per-expert outputs
with the routing weights.

Device kernel (per core), transposed layout (no on-device transposes):
  warmup : ~14 matmuls on a zeroed SBUF tile run during the initial DMA
           wait so the PE HAM clock-gate reaches 8/8 (2.4 GHz) before the
           first real matmul; the real stream then runs fully warm.
  stage 1: guT[2816, C] = w13 @ xT   (per row-block m: g chain then u
           chain over 16 K-tiles; x arrives in 4 chunk DMAs sized so the
           chains stall minimally behind the HBM supply)
  stage 2: actT[1408, C] = silu(gT) * uT   (ScalarE Silu + VectorE mul)
  stage 3: yT[2048, C] = w2 @ actT, PSUM->SBUF copy on VectorE as fp16
           (halves the output DMA bytes; adds ~1e-4 rel-err)

Matmuls in fp16 (full PE rate, half the DMA bytes of fp32; fp32 PSUM
accumulation keeps rel-err ~6e-4). DMA plan: first row-block's weights as
quarter transfers interleaved with the x chunks (ordered by first-use so
the ~0.65us per-dma_start HWDGE issue cost and the ~358GB/s HBM supply
gate the stream as little as possible); later w13 row-blocks as single
1MB merged g+u transfers; w2 as four-row-block pairs prefetched during
stage 3 itself so they never steal bandwidth from the stage-1 stream.
"""

import sys

if "/opt/trn_rl_repo" not in sys.path:
    sys.path.insert(0, "/opt/trn_rl_repo")

import os
import numpy as np
from contextlib import ExitStack

import concourse.bass as bass
import concourse.tile as tile
from concourse import bacc, mybir

T, H, I, E, K = 2048, 2048, 1408, 8, 2
CMAX = 512                   # max token capacity per expert per pass (PSUM bank)
HT = H // 128                # 16 K-tiles over H
IT = I // 128                # 11 K-tiles over I
BT = 2 * I // 128            # 22 row-blocks of guT
XSPLIT = (1, 3, 4, 4, 4)     # x chunk sizes in k-tiles (first-use ordered)
NWARM = 10                   # PE warmup matmuls (fill the DMA head, warm HAM)

DT = mybir.dt.float16
NP_DT = np.float16
F32 = mybir.dt.float32

_cache: dict = {}


def _build_nc(C):
    """Build + compile the per-core FFN program (same program on all cores)."""
    nc = bacc.Bacc("TRN2", target_bir_lowering=False, debug=False, num_devices=E)
    # x: [128, HT*C], k-tiles side by side; chunk DMAs take column ranges
    x_d = nc.dram_tensor("x_sb", [128, HT * C], DT, kind="ExternalInput")
    # w13 row m: [wg_m | wu_m], each [128, HT*128]
    w13_d = nc.dram_tensor("w13_sb", [IT, 128, 2 * HT * 128], DT, kind="ExternalInput")
    # w2 pair p: [block 2p | block 2p+1], each [128, IT*128]
    w2_d = nc.dram_tensor("w2_sb", [HT // 2, 128, 2 * IT * 128], DT, kind="ExternalInput")
    y_d = nc.dram_tensor("y_sb", [HT, 128, C], DT, kind="ExternalOutput")

    AF = mybir.ActivationFunctionType
    GW = HT * 128  # columns of one g (or u) row-block

    with tile.TileContext(nc) as tc, ExitStack() as ctx:
        zp = ctx.enter_context(tc.tile_pool(name="z", bufs=1))
        xp = ctx.enter_context(tc.tile_pool(name="x", bufs=1))
        w0p = ctx.enter_context(tc.tile_pool(name="w0", bufs=1))
        wp = ctx.enter_context(tc.tile_pool(name="w", bufs=6))
        w2p = ctx.enter_context(tc.tile_pool(name="w2", bufs=2))
        ap = ctx.enter_context(tc.tile_pool(name="act", bufs=1))
        sp = ctx.enter_context(tc.tile_pool(name="tmp", bufs=2))
        yp = ctx.enter_context(tc.tile_pool(name="y", bufs=3))
        psg = ctx.enter_context(
            tc.tile_pool(name="psg", bufs=5, space=bass.MemorySpace.PSUM)
        )
        psy = ctx.enter_context(
            tc.tile_pool(name="psy", bufs=3, space=bass.MemorySpace.PSUM)
        )

        # --- PE warmup: no DMA deps, runs during the initial DMA wait -----
        zw = zp.tile([128, 128], DT, tag="zw")
        nc.gpsimd.memset(zw[:], 0.0)
        zx = zp.tile([128, C], DT, tag="zx")
        nc.gpsimd.memset(zx[:], 0.0)
        warm_ps = psg.tile([128, C], F32, tag="ps")
        for i in range(NWARM):
            nc.tensor.matmul(warm_ps[:], zw[:], zx[:], start=True, stop=True)

        # --- DMA issue schedule (ordered by first-use time) ----------------
        # m=0/halves of m=1,2 weights as half transfers interleaved with
        # the x chunks; x chunk 0 is a single k-tile so the first matmul's
        # deps are tiny. All on the sync HWDGE ring, in consumption order.
        w0 = {}
        def _load_w0(which, half):
            # which: 0=g, 1=u; half: 0 = k-tiles 0..7, 1 = k-tiles 8..15
            t = w0p.tile([128, GW // 2], DT, tag=f"w0_{which}_{half}")
            src = w13_d.ap()[0][:, which * GW + half * (GW // 2):
                                which * GW + (half + 1) * (GW // 2)]
            nc.sync.dma_start(t[:], src)
            w0[(which, half)] = t

        x_t = []      # (tile, k_start, n_k)
        def _load_x(q):
            k0 = sum(XSPLIT[:q])
            nk = XSPLIT[q]
            xt = xp.tile([128, nk * C], DT, tag=f"x{q}")
            # scalar HWDGE ring: x transfers and their completion sems run
            # in parallel with the weight stream on the sync ring
            nc.scalar.dma_start(xt[:], x_d.ap()[:, k0 * C: (k0 + nk) * C])
            x_t.append((xt, k0, nk))

        wgu = {}
        def _load_w13_half(m, which):
            # one g (which=0) or u (which=1) row-block of w13, 512KB
            t = wp.tile([128, GW], DT, tag="w13")
            nc.sync.dma_start(t[:], w13_d.ap()[m][:, which * GW: (which + 1) * GW])
            wgu[(m, which)] = t

        _load_w0(0, 0)   # wg k0-7: first matmul dep
        _load_x(0)       # k0
        _load_x(1)       # k1-3
        _load_x(2)       # k4-7
        _load_w0(1, 0)   # wu k0-7
        _load_x(3)       # k8-11
        _load_w0(0, 1)   # wg k8-15
        _load_x(4)       # k12-15
        _load_w0(1, 1)   # wu k8-15
        _load_w13_half(1, 0)
        _load_w13_half(1, 1)
        _load_w13_half(2, 0)
        _load_w13_half(2, 1)

        def xsl(k):
            for xt, k0, nk in x_t:
                if k0 <= k < k0 + nk:
                    return xt[:, (k - k0) * C: (k - k0 + 1) * C]
            raise AssertionError(k)

        # --- stage 1 + 2 ---------------------------------------------------
        act_t = []
        for m in range(IT):
            if m == 0:
                g_w = [w0[(0, k // 8)][:, (k % 8) * 128: (k % 8 + 1) * 128]
                       for k in range(HT)]
                u_w = [w0[(1, k // 8)][:, (k % 8) * 128: (k % 8 + 1) * 128]
                       for k in range(HT)]
            else:
                for which in (0, 1):
                    if (m, which) not in wgu:
                        _load_w13_half(m, which)
                g_t = wgu.pop((m, 0))
                u_t = wgu.pop((m, 1))
                g_w = [g_t[:, k * 128: (k + 1) * 128] for k in range(HT)]
                u_w = [u_t[:, k * 128: (k + 1) * 128] for k in range(HT)]
            g_ps = psg.tile([128, C], F32, tag="ps")
            u_ps = psg.tile([128, C], F32, tag="ps")
            if m == 0:
                # k-half split: halves the early x bandwidth demand so the
                # chains track the HBM supply instead of outrunning it 2x
                for k in range(HT // 2):
                    nc.tensor.matmul(g_ps[:], g_w[k], xsl(k),
                                     start=(k == 0), stop=False)
                for k in range(HT // 2):
                    nc.tensor.matmul(u_ps[:], u_w[k], xsl(k),
                                     start=(k == 0), stop=False)
                for k in range(HT // 2, HT):
                    nc.tensor.matmul(g_ps[:], g_w[k], xsl(k),
                                     start=False, stop=(k == HT - 1))
                for k in range(HT // 2, HT):
                    nc.tensor.matmul(u_ps[:], u_w[k], xsl(k),
                                     start=False, stop=(k == HT - 1))
            else:
                for k in range(HT):
                    nc.tensor.matmul(g_ps[:], g_w[k], xsl(k),
                                     start=(k == 0), stop=(k == HT - 1))
                for k in range(HT):
                    nc.tensor.matmul(u_ps[:], u_w[k], xsl(k),
                                     start=(k == 0), stop=(k == HT - 1))
            sg = sp.tile([128, C], F32, tag="sg")
            nc.scalar.activation(sg[:], g_ps[:], AF.Silu)
            at = ap.tile([128, C], DT, tag=f"act{m}")
            nc.vector.tensor_mul(at[:], sg[:], u_ps[:])
            act_t.append(at)

        # --- stage 3 -------------------------------------------------------
        # w2 pair p is DMA'd two blocks ahead of first use (pairs 0,1 queue
        # behind the last w13 transfers; later pairs issue inside the loop).
        w2t = {}
        def _load_w2(p):
            t = w2p.tile([128, 2 * IT * 128], DT, tag=f"w2_{p % 2}")
            nc.sync.dma_start(t[:], w2_d.ap()[p])
            w2t[p] = t

        _load_w2(0)
        _load_w2(1)
        for m in range(HT):
            p = m // 2
            if m % 2 == 0 and p + 2 <= HT // 2 - 1:
                _load_w2(p + 2)
            wt = w2t[p]
            base = (m % 2) * IT * 128
            y_ps = psy.tile([128, C], F32, tag="y")
            for k in range(IT):
                nc.tensor.matmul(
                    y_ps[:], wt[:, base + k * 128: base + (k + 1) * 128],
                    act_t[k][:], start=(k == 0), stop=(k == IT - 1),
                )
            if m % 2 == 1:
                del w2t[p]
            y_sb = yp.tile([128, C], DT, tag="yout")
            nc.vector.tensor_copy(y_sb[:], y_ps[:])
            # scalar HWDGE ring: keeps y stores off the sync ring's queue
            nc.scalar.dma_start(y_d.ap()[m], y_sb[:])

    nc.compile()
    return nc


def _get_nc(C):
    if C not in _cache:
        _cache[C] = _build_nc(C)
    return _cache[C]


def _prep_weights(w13, w2):
    """Pre-tile weights into the SBUF layout the kernel DMAs verbatim."""
    wb = (
        w13.reshape(E, BT, 128, HT, 128)
        .transpose(0, 1, 4, 3, 2)
        .astype(NP_DT)
        .reshape(E, BT, 128, HT * 128)
    )
    # row m of w13_sb = concat(g block m, u block m+IT)
    w13_sb = np.concatenate([wb[:, :IT], wb[:, IT:]], axis=3)
    w2b = (
        w2.reshape(E, HT, 128, IT, 128)
        .transpose(0, 1, 4, 3, 2)
        .astype(NP_DT)
        .reshape(E, HT, 128, IT * 128)
    )
    w2_sb = np.concatenate([w2b[:, 0::2], w2b[:, 1::2]], axis=3)
    return np.ascontiguousarray(w13_sb), np.ascontiguousarray(w2_sb)


def _prep_x(xe, C):
    """[C, H] fp32 -> [128, HT*C] fp16, k-tiles side by side."""
    xt = xe.T.reshape(HT, 128, C).transpose(1, 0, 2)
    return np.ascontiguousarray(xt).reshape(128, HT * C).astype(NP_DT)


def kernel(
    hidden_states,
    topk_weights,
    topk_ids,
    w13,
    w2,
    num_global_tokens=None,
    max_num_tokens_per_gpu=None,
):
    from concourse.bass_utils import run_bass_kernel_spmd

    hs = np.asarray(hidden_states, dtype=np.float32)
    tw = np.asarray(topk_weights, dtype=np.float32)
    ti = np.asarray(topk_ids)
    w13 = np.asarray(w13, dtype=np.float32)
    w2 = np.asarray(w2, dtype=np.float32)

    assert hs.shape == (T, H), hs.shape
    assert w13.shape == (E, 2 * I, H), w13.shape
    assert w2.shape == (E, H, I), w2.shape

    # per-(token, expert) combine weights: sum of topk weights routed to e
    # (out-of-range ids contribute nothing, matching jax.nn.one_hot)
    comb = np.zeros((T, E), dtype=np.float32)
    for k in range(ti.shape[1]):
        col = ti[:, k]
        ok = (col >= 0) & (col < E)
        np.add.at(comb, (np.arange(T)[ok], col[ok]), tw[ok, k])

    idxs = [np.nonzero(comb[:, e])[0] for e in range(E)]
    need = max(len(ix) for ix in idxs)
    # token capacity: matmul N dim, multiple of 4 (x chunk layout)
    C = min(CMAX, max(64, -(-need // 4) * 4))
    nchunks = max(1, -(-need // C))

    w13_sb, w2_sb = _prep_weights(w13, w2)
    nc = _get_nc(C)

    trace = bool(os.environ.get("KERNEL_PROFILE"))
    out = np.zeros((T, H), dtype=np.float32)
    for chunk in range(nchunks):
        in_maps = []
        sels = []
        for e in range(E):
            sel = idxs[e][chunk * C: (chunk + 1) * C]
            xe = np.zeros((C, H), dtype=np.float32)
            xe[: len(sel)] = hs[sel]
            in_maps.append(
                {"x_sb": _prep_x(xe, C), "w13_sb": w13_sb[e], "w2_sb": w2_sb[e]}
            )
            sels.append(sel)
        if trace:
            try:
                res = run_bass_kernel_spmd(nc, in_maps, list(range(E)), trace=True)
                if res.exec_time_ns is not None:
                    print(f"HW exec time: {res.exec_time_ns} ns")
            except Exception:
                res = run_bass_kernel_spmd(nc, in_maps, list(range(E)))
        else:
            res = run_bass_kernel_spmd(nc, in_maps, list(range(E)))
        for e in range(E):
            sel = sels[e]
            if len(sel) == 0:
                continue
            y_sb = np.asarray(res.results[e]["y_sb"], dtype=np.float32)
            ye = y_sb.reshape(H, C).T  # [C, H]
            out[sel] += comb[sel, e][:, None] * ye[: len(sel)]
    return out
